# revision 4
# baseline (speedup 1.0000x reference)
"""Trainium2 Bass kernel for nn_ConvShare: multi-width causal conv + shared projection.

Reference computation (per batch element b):
    xpad = pad(x[b], L -> L+W-1)                       # [L+11, D]
    taps[k]  = xpad[k:k+L, :] @ conv_w[:, :, k].T      # [L, D], k = 0..W-1
    spans[k] = cumsum_k taps                           # [L, D]
    h[k]     = relu(spans[k])
    out[:, k, :] = h[k] @ proj_w.T + proj_b            # [L, W, D]

Sharding: data-parallel over batch B=8 across the 8 NeuronCores (no
communication; conv_w/proj_w replicated per core).

Everything on-chip is feature-major ([D, L], contraction dim on SBUF
partitions). fp16 matmul inputs (PSUM accumulation fp32; conv cumsum
carried in fp32 SBUF). The PE roofline is 24 unit-matmuls of
512x768x768 = 184.3us; this implementation closes in on it with:
  - PE warm-up: dummy N=32 matmuls with no DMA deps run during the
    startup input DMA so the HAM clock gate (1.2 -> 2.4 GHz) opens
    before the first real matmul.
  - c-outer conv waves + finely split tap-0 DMAs (xT in 4 partition
    slices, tap-0 conv weights in 36 [128,128] pieces) so the first
    conv wave needs only ~330KB of DMA instead of ~2MB.
  - Batched DMA everywhere else: one 9KB-per-partition transfer per
    conv-weight tap, one 6KB-per-partition transfer per output tap
    (128 lines instead of 768). The DMA engines are line-issue-rate
    limited (~0.1-0.2us per line per queue), not bandwidth limited.
  - fp16 output, transposed/upcast to [L, W, D] host-side.
  - The last tap is emitted in two l-halves and its output DMA in two
    o2-halves so the final drain chain is short.
"""

import os
import sys

import numpy as np

if True:  # make concourse importable regardless of harness cwd
    for _p in ("/opt/trn_rl_repo", "/opt/pypackages"):
        if _p not in sys.path and os.path.isdir(_p):
            sys.path.append(_p)

from contextlib import ExitStack  # noqa: E402

import concourse.bacc as bacc  # noqa: E402
import concourse.bass as bass  # noqa: E402
import concourse.mybir as mybir  # noqa: E402
import concourse.tile as tile  # noqa: E402
from concourse import bass_utils  # noqa: E402

B, L, D, W = 8, 512, 768, 12
P = 128          # SBUF partitions
C = D // P       # 6 contraction chunks of 128
LP = L + W - 1   # 523: right-padded sequence length

F32 = mybir.dt.float32
F16 = mybir.dt.float16
RELU = mybir.ActivationFunctionType.Relu

WARMUP = 64      # dummy N=32 matmuls to open the HAM clock gate during startup DMA
SPLIT_LAST = True  # emit tap W-1 in two l-halves for a faster tail drain

# Knobs the test harness may flip before calling kernel():
TRACE = False
LAST_RESULTS = None


def _build_program(warmup: int, split_last: bool) -> bass.Bass:
    mdt = F16

    nc = bacc.Bacc(
        "TRN2",
        target_bir_lowering=False,
        debug=False,
        num_devices=B,
    )

    # DRAM I/O (pre-arranged host-side so every DMA is a clean copy with
    # large contiguous per-partition lines).
    xT = nc.dram_tensor("xT", [C, P, LP], mdt, kind="ExternalInput").ap()
    cw0s = nc.dram_tensor("cw0s", [C, C, P, P], mdt, kind="ExternalInput").ap()
    cwB = nc.dram_tensor("cwB", [W, P, C * D], mdt, kind="ExternalInput").ap()
    pwB = nc.dram_tensor("pwB", [P, C * D], mdt, kind="ExternalInput").ap()
    pb = nc.dram_tensor("pb", [C, P, 1], F32, kind="ExternalInput").ap()
    out = nc.dram_tensor("out", [W, P, C, L], F16, kind="ExternalOutput").ap()

    with tile.TileContext(nc) as tc, ExitStack() as ctx:
        const_pool = ctx.enter_context(tc.tile_pool(name="const", bufs=1))
        cw_pool = ctx.enter_context(tc.tile_pool(name="cw", bufs=2))
        h_pool = ctx.enter_context(tc.tile_pool(name="h", bufs=2))
        out_pool = ctx.enter_context(tc.tile_pool(name="out", bufs=2))
        psc_pool = ctx.enter_context(tc.tile_pool(name="psc", bufs=1, space="PSUM"))
        psp_pool = ctx.enter_context(tc.tile_pool(name="psp", bufs=2, space="PSUM"))

        # --- PE warm-up: no-data-dependency matmuls that run while the first
        # input DMAs are in flight, so the HAM gate opens (~3.4us of PE busy)
        # before the real stream starts.
        if warmup:
            wa = const_pool.tile([P, P], mdt, name="warm_a")
            wb = const_pool.tile([P, 32], mdt, name="warm_b")
            nc.gpsimd.memset(wa[:], 0.0)
            nc.gpsimd.memset(wb[:], 0.0)
            for wi in range(warmup):
                wp = psp_pool.tile([P, 512], F32, tag="psp", name=f"warm_ps{wi}")
                nc.tensor.matmul(
                    wp[:, 0:32], lhsT=wa[:], rhs=wb[:], start=True, stop=True
                )

        # --- running conv cumsum, fp32 (memset overlaps startup DMA)
        spans = const_pool.tile([P, C * L], F32)
        nc.gpsimd.memset(spans[:], 0.0)

        # --- startup DMAs in "wave" order: the tap-0 conv runs c-outer, so
        # wave c needs only xT[c] (4 partition-sliced DMAs) + 6 small weight
        # pieces. pw/pb/cw[1] follow after tap-0's needs.
        xT_t = []
        cw0_t = []  # [c][ob] -> [128,128] tile
        for c in range(C):
            xt = const_pool.tile([P, LP], mdt, tag=f"xt{c}", name=f"xt{c}")
            for p0 in range(0, P, 32):
                nc.sync.dma_start(xt[p0 : p0 + 32, :], xT[c, p0 : p0 + 32, :])
            xT_t.append(xt)
            obs = []
            for ob in range(C):
                t = const_pool.tile([P, P], mdt, tag=f"cw0_{c}_{ob}", name=f"cw0_{c}_{ob}")
                nc.sync.dma_start(t[:], cw0s[c, ob, :, :])
                obs.append(t)
            cw0_t.append(obs)

        def load_cw(k):
            # one DMA per tap: 128 lines of 9KB
            t = cw_pool.tile([P, C * D], mdt, tag="cw", name=f"cw_{k}")
            nc.sync.dma_start(t[:], cwB[k, :, :])
            return t

        # conv psum banks: one persistent tag per output block (6 banks; the
        # remaining 2 banks cycle for the proj).
        ps = [
            psc_pool.tile([P, L], F32, tag=f"sp{ob}", name=f"sp{ob}")
            for ob in range(C)
        ]

        def conv_tap(k, cw_k, l0, ln):
            # wave order: all 6 output blocks for contraction chunk c, then c+1.
            for c in range(C):
                for ob in range(C):
                    lhsT = (
                        cw0_t[c][ob][:]
                        if k == 0
                        else cw_k[:, c * D + ob * P : c * D + (ob + 1) * P]
                    )
                    nc.tensor.matmul(
                        ps[ob][:, l0 : l0 + ln],
                        lhsT=lhsT,
                        rhs=xT_t[c][:, k + l0 : k + l0 + ln],
                        start=(c == 0),
                        stop=(c == C - 1),
                        skip_group_check=True,
                    )

        def relu_tap(h_t, l0, ln):
            for ob in range(C):
                sp = spans[:, ob * L + l0 : ob * L + l0 + ln]
                nc.vector.tensor_add(sp, sp, ps[ob][:, l0 : l0 + ln])  # cumsum
                nc.scalar.activation(h_t[ob][:, l0 : l0 + ln], sp, RELU)

        def proj_tap(k, h_t, pw_t, pb_t, o_tap, l0, ln):
            for o2b in range(C):
                pp = psp_pool.tile([P, 512], F32, tag="psp", name=f"pp_{k}_{o2b}_{l0}")
                for c in range(C):
                    nc.tensor.matmul(
                        pp[:, 0:ln],
                        lhsT=pw_t[:, c * D + o2b * P : c * D + (o2b + 1) * P],
                        rhs=h_t[c][:, l0 : l0 + ln],
                        start=(c == 0),
                        stop=(c == C - 1),
                    )
                nc.vector.tensor_scalar_add(
                    o_tap[:, o2b * L + l0 : o2b * L + l0 + ln], pp[:, 0:ln], pb_t[o2b][:]
                )

        # --- tap 0 conv first (its matmuls only need the wave DMAs above) ---
        conv_tap(0, None, 0, L)

        # remaining startup loads, ordered by first use
        cw_cur = load_cw(1)
        pw_t = const_pool.tile([P, C * D], mdt, name="pw")
        nc.sync.dma_start(pw_t[:], pwB[:, :])
        pb_t = []
        for c in range(C):
            t = const_pool.tile([P, 1], F32, tag=f"pb{c}", name=f"pb{c}")
            nc.sync.dma_start(t[:], pb[c, :, :])
            pb_t.append(t)

        h_t = [h_pool.tile([P, L], mdt, tag=f"h{c}", name=f"h{c}_0") for c in range(C)]
        o_tap = out_pool.tile([P, C * L], F16, tag="out", name="o_0")
        relu_tap(h_t, 0, L)
        proj_tap(0, h_t, pw_t, pb_t, o_tap, 0, L)
        nc.sync.dma_start(out[0, :, :, :], o_tap[:])

        for k in range(1, W):
            cw_k = cw_cur
            cw_cur = load_cw(k + 1) if k + 1 < W else None
            h_t = [
                h_pool.tile([P, L], mdt, tag=f"h{c}", name=f"h{c}_{k}")
                for c in range(C)
            ]
            o_tap = out_pool.tile([P, C * L], F16, tag="out", name=f"o_{k}")
            conv_tap(k, cw_k, 0, L)
            relu_tap(h_t, 0, L)
            if split_last and k == W - 1:
                # per-o2b output DMAs so each drains while later proj groups
                # still compute; the final DMA is only 128x1KB.
                for o2b in range(C):
                    pp = psp_pool.tile([P, 512], F32, tag="psp", name=f"pp_{k}_{o2b}")
                    for c in range(C):
                        nc.tensor.matmul(
                            pp[:],
                            lhsT=pw_t[:, c * D + o2b * P : c * D + (o2b + 1) * P],
                            rhs=h_t[c][:],
                            start=(c == 0),
                            stop=(c == C - 1),
                        )
                    nc.vector.tensor_scalar_add(
                        o_tap[:, o2b * L : (o2b + 1) * L], pp[:], pb_t[o2b][:]
                    )
                    nc.sync.dma_start(
                        out[k, :, o2b, :], o_tap[:, o2b * L : (o2b + 1) * L]
                    )
            else:
                proj_tap(k, h_t, pw_t, pb_t, o_tap, 0, L)
                nc.sync.dma_start(out[k, :, :, :], o_tap[:])

    nc.compile()
    return nc


_program_cache: dict = {}


def _get_program() -> bass.Bass:
    key = (WARMUP, SPLIT_LAST)
    if key not in _program_cache:
        _program_cache[key] = _build_program(WARMUP, SPLIT_LAST)
    return _program_cache[key]


def _prep_inputs(x, conv_w, proj_w, proj_b):
    x = np.asarray(x, dtype=np.float32)
    conv_w = np.asarray(conv_w, dtype=np.float32)
    proj_w = np.asarray(proj_w, dtype=np.float32)
    proj_b = np.asarray(proj_b, dtype=np.float32)

    xT_all = np.zeros((B, D, LP), dtype=np.float32)              # [B, D, L+W-1]
    xT_all[:, :, :L] = x.transpose(0, 2, 1)
    xT_all = np.ascontiguousarray(xT_all.reshape(B, C, P, LP).astype(np.float16))
    cwT = conv_w.transpose(2, 1, 0).reshape(W, C, P, D).astype(np.float16)
    # tap-0 weights additionally pre-split into 36 [128,128] pieces
    cw0s = np.ascontiguousarray(
        cwT[0].reshape(C, P, C, P).transpose(0, 2, 1, 3)
    )                                                            # [c, ob, P, P]
    # batched per-tap layout: cwB[k, p, c*D + o] = conv_w[o, c*128+p, k]
    cwB = np.ascontiguousarray(cwT.transpose(0, 2, 1, 3).reshape(W, P, C * D))
    pwT = proj_w.T.reshape(C, P, D).astype(np.float16)
    pwB = np.ascontiguousarray(pwT.transpose(1, 0, 2).reshape(P, C * D))
    pbb = np.ascontiguousarray(proj_b.reshape(C, P, 1))
    return xT_all, cw0s, cwB, pwB, pbb


def kernel(x, conv_w, proj_w, proj_b):
    global LAST_RESULTS
    nc = _get_program()
    xT_all, cw0s, cwB, pwB, pbb = _prep_inputs(x, conv_w, proj_w, proj_b)
    in_maps = [
        {"xT": xT_all[b], "cw0s": cw0s, "cwB": cwB, "pwB": pwB, "pb": pbb}
        for b in range(B)
    ]
    res = bass_utils.run_bass_kernel_spmd(
        nc, in_maps, core_ids=list(range(B)), trace=TRACE
    )
    LAST_RESULTS = res
    # per-core out is [W, P, C, L] f16; final layout is [L, W, D] with
    # D = c*128 + p
    return np.stack(
        [
            np.ascontiguousarray(
                r["out"].astype(np.float32).transpose(3, 0, 2, 1).reshape(L, W, D)
            )
            for r in res.results
        ],
        axis=0,
    )


# revision 5
# speedup vs baseline: 1.0077x; 1.0077x over previous
"""Trainium2 Bass kernel for nn_ConvShare: multi-width causal conv + shared projection.

Reference computation (per batch element b):
    xpad = pad(x[b], L -> L+W-1)                       # [L+11, D]
    taps[k]  = xpad[k:k+L, :] @ conv_w[:, :, k].T      # [L, D], k = 0..W-1
    spans[k] = cumsum_k taps                           # [L, D]
    h[k]     = relu(spans[k])
    out[:, k, :] = h[k] @ proj_w.T + proj_b            # [L, W, D]

Sharding: data-parallel over batch B=8 across the 8 NeuronCores (no
communication; conv_w/proj_w replicated per core).

Everything on-chip is feature-major ([D, L], contraction dim on SBUF
partitions). fp16 matmul inputs (PSUM accumulation fp32; conv cumsum
carried in fp32 SBUF). The PE roofline is 24 unit-matmuls of
512x768x768 = 184.3us; this implementation closes in on it with:
  - PE warm-up: dummy N=32 matmuls with no DMA deps run during the
    startup input DMA so the HAM clock gate (1.2 -> 2.4 GHz) opens
    before the first real matmul.
  - c-outer conv waves + finely split tap-0 DMAs (xT in 4 partition
    slices, tap-0 conv weights in 36 [128,128] pieces) so the first
    conv wave needs only ~330KB of DMA instead of ~2MB.
  - Batched DMA everywhere else: one 9KB-per-partition transfer per
    conv-weight tap, one 6KB-per-partition transfer per output tap
    (128 lines instead of 768). The DMA engines are line-issue-rate
    limited (~0.1-0.2us per line per queue), not bandwidth limited.
  - fp16 output, transposed/upcast to [L, W, D] host-side.
  - The last tap is emitted in two l-halves and its output DMA in two
    o2-halves so the final drain chain is short.
"""

import os
import sys

import numpy as np

if True:  # make concourse importable regardless of harness cwd
    for _p in ("/opt/trn_rl_repo", "/opt/pypackages"):
        if _p not in sys.path and os.path.isdir(_p):
            sys.path.append(_p)

from contextlib import ExitStack  # noqa: E402

import concourse.bacc as bacc  # noqa: E402
import concourse.bass as bass  # noqa: E402
import concourse.mybir as mybir  # noqa: E402
import concourse.tile as tile  # noqa: E402
from concourse import bass_utils  # noqa: E402

B, L, D, W = 8, 512, 768, 12
P = 128          # SBUF partitions
C = D // P       # 6 contraction chunks of 128
LP = L + W - 1   # 523: right-padded sequence length

F32 = mybir.dt.float32
F16 = mybir.dt.float16
RELU = mybir.ActivationFunctionType.Relu

WARMUP = 64      # dummy N=32 matmuls to open the HAM clock gate during startup DMA
SPLIT_LAST = True  # emit tap W-1 in two l-halves for a faster tail drain

# Knobs the test harness may flip before calling kernel():
TRACE = False
LAST_RESULTS = None


def _build_program(warmup: int, split_last: bool) -> bass.Bass:
    mdt = F16

    nc = bacc.Bacc(
        "TRN2",
        target_bir_lowering=False,
        debug=False,
        num_devices=B,
    )

    # DRAM I/O (pre-arranged host-side so every DMA is a clean copy with
    # large contiguous per-partition lines).
    xT = nc.dram_tensor("xT", [C, P, LP], mdt, kind="ExternalInput").ap()
    cw0s = nc.dram_tensor("cw0s", [C, C, P, P], mdt, kind="ExternalInput").ap()
    cwB = nc.dram_tensor("cwB", [W, P, C * D], mdt, kind="ExternalInput").ap()
    pwB = nc.dram_tensor("pwB", [P, C * D], mdt, kind="ExternalInput").ap()
    pb = nc.dram_tensor("pb", [C, P, 1], F32, kind="ExternalInput").ap()
    out = nc.dram_tensor("out", [W, P, C, L], F16, kind="ExternalOutput").ap()

    with tile.TileContext(nc) as tc, ExitStack() as ctx:
        const_pool = ctx.enter_context(tc.tile_pool(name="const", bufs=1))
        cw_pool = ctx.enter_context(tc.tile_pool(name="cw", bufs=2))
        h_pool = ctx.enter_context(tc.tile_pool(name="h", bufs=2))
        out_pool = ctx.enter_context(tc.tile_pool(name="out", bufs=2))
        psc_pool = ctx.enter_context(tc.tile_pool(name="psc", bufs=1, space="PSUM"))
        psp_pool = ctx.enter_context(tc.tile_pool(name="psp", bufs=2, space="PSUM"))

        # --- PE warm-up: no-data-dependency matmuls that run while the first
        # input DMAs are in flight, so the HAM gate opens (~3.4us of PE busy)
        # before the real stream starts.
        if warmup:
            wa = const_pool.tile([P, P], mdt, name="warm_a")
            wb = const_pool.tile([P, 32], mdt, name="warm_b")
            nc.gpsimd.memset(wa[:], 0.0)
            nc.gpsimd.memset(wb[:], 0.0)
            for wi in range(warmup):
                wp = psp_pool.tile([P, 512], F32, tag="psp", name=f"warm_ps{wi}")
                nc.tensor.matmul(
                    wp[:, 0:32], lhsT=wa[:], rhs=wb[:], start=True, stop=True
                )

        # --- running conv cumsum, fp32 (memset overlaps startup DMA)
        spans = const_pool.tile([P, C * L], F32)
        nc.gpsimd.memset(spans[:], 0.0)

        # --- startup DMAs in "wave" order: the tap-0 conv runs c-outer, so
        # wave c needs only xT[c] (4 partition-sliced DMAs) + 6 small weight
        # pieces. pw/pb/cw[1] follow after tap-0's needs.
        xT_t = []
        cw0_t = []  # [c][ob] -> [128,128] tile
        for c in range(C):
            xt = const_pool.tile([P, LP], mdt, tag=f"xt{c}", name=f"xt{c}")
            for p0 in range(0, P, 32):
                nc.sync.dma_start(xt[p0 : p0 + 32, :], xT[c, p0 : p0 + 32, :])
            xT_t.append(xt)
            obs = []
            for ob in range(C):
                t = const_pool.tile([P, P], mdt, tag=f"cw0_{c}_{ob}", name=f"cw0_{c}_{ob}")
                nc.sync.dma_start(t[:], cw0s[c, ob, :, :])
                obs.append(t)
            cw0_t.append(obs)

        def load_cw(k):
            # one DMA per tap: 128 lines of 9KB
            t = cw_pool.tile([P, C * D], mdt, tag="cw", name=f"cw_{k}")
            nc.sync.dma_start(t[:], cwB[k, :, :])
            return t

        # conv psum banks: one persistent tag per output block (6 banks; the
        # remaining 2 banks cycle for the proj).
        ps = [
            psc_pool.tile([P, L], F32, tag=f"sp{ob}", name=f"sp{ob}")
            for ob in range(C)
        ]

        def conv_tap(k, cw_k, l0, ln):
            # wave order: all 6 output blocks for contraction chunk c, then c+1.
            for c in range(C):
                for ob in range(C):
                    lhsT = (
                        cw0_t[c][ob][:]
                        if k == 0
                        else cw_k[:, c * D + ob * P : c * D + (ob + 1) * P]
                    )
                    nc.tensor.matmul(
                        ps[ob][:, l0 : l0 + ln],
                        lhsT=lhsT,
                        rhs=xT_t[c][:, k + l0 : k + l0 + ln],
                        start=(c == 0),
                        stop=(c == C - 1),
                        skip_group_check=True,
                    )

        def relu_tap(h_t, l0, ln):
            for ob in range(C):
                sp = spans[:, ob * L + l0 : ob * L + l0 + ln]
                nc.vector.tensor_add(sp, sp, ps[ob][:, l0 : l0 + ln])  # cumsum
                nc.scalar.activation(h_t[ob][:, l0 : l0 + ln], sp, RELU)

        def proj_tap(k, h_t, pw_t, pb_t, o_tap, l0, ln):
            for o2b in range(C):
                pp = psp_pool.tile([P, 512], F32, tag="psp", name=f"pp_{k}_{o2b}_{l0}")
                for c in range(C):
                    nc.tensor.matmul(
                        pp[:, 0:ln],
                        lhsT=pw_t[:, c * D + o2b * P : c * D + (o2b + 1) * P],
                        rhs=h_t[c][:, l0 : l0 + ln],
                        start=(c == 0),
                        stop=(c == C - 1),
                    )
                nc.vector.tensor_scalar_add(
                    o_tap[:, o2b * L + l0 : o2b * L + l0 + ln], pp[:, 0:ln], pb_t[o2b][:]
                )

        # --- tap 0 conv first (its matmuls only need the wave DMAs above) ---
        conv_tap(0, None, 0, L)

        # remaining startup loads, ordered by first use
        cw_cur = load_cw(1)
        pw_t = const_pool.tile([P, C * D], mdt, name="pw")
        nc.sync.dma_start(pw_t[:], pwB[:, :])
        pb_t = []
        for c in range(C):
            t = const_pool.tile([P, 1], F32, tag=f"pb{c}", name=f"pb{c}")
            nc.sync.dma_start(t[:], pb[c, :, :])
            pb_t.append(t)

        h_t = [h_pool.tile([P, L], mdt, tag=f"h{c}", name=f"h{c}_0") for c in range(C)]
        o_tap = out_pool.tile([P, C * L], F16, tag="out", name="o_0")
        relu_tap(h_t, 0, L)
        proj_tap(0, h_t, pw_t, pb_t, o_tap, 0, L)
        nc.sync.dma_start(out[0, :, :, :], o_tap[:])

        for k in range(1, W):
            cw_k = cw_cur
            cw_cur = load_cw(k + 1) if k + 1 < W else None
            h_t = [
                h_pool.tile([P, L], mdt, tag=f"h{c}", name=f"h{c}_{k}")
                for c in range(C)
            ]
            o_tap = out_pool.tile([P, C * L], F16, tag="out", name=f"o_{k}")
            conv_tap(k, cw_k, 0, L)
            relu_tap(h_t, 0, L)
            if split_last and k == W - 1:
                # output in two o2-half DMAs (3KB lines): the first drains
                # while proj groups 3-5 still compute.
                for o2b in range(C):
                    pp = psp_pool.tile([P, 512], F32, tag="psp", name=f"pp_{k}_{o2b}")
                    for c in range(C):
                        nc.tensor.matmul(
                            pp[:],
                            lhsT=pw_t[:, c * D + o2b * P : c * D + (o2b + 1) * P],
                            rhs=h_t[c][:],
                            start=(c == 0),
                            stop=(c == C - 1),
                        )
                    nc.vector.tensor_scalar_add(
                        o_tap[:, o2b * L : (o2b + 1) * L], pp[:], pb_t[o2b][:]
                    )
                    if o2b == 2:
                        nc.sync.dma_start(out[k, :, 0:3, :], o_tap[:, 0 : 3 * L])
                    elif o2b == 5:
                        nc.sync.dma_start(out[k, :, 3:6, :], o_tap[:, 3 * L : 6 * L])
            else:
                proj_tap(k, h_t, pw_t, pb_t, o_tap, 0, L)
                nc.sync.dma_start(out[k, :, :, :], o_tap[:])

    nc.compile()
    return nc


_program_cache: dict = {}


def _get_program() -> bass.Bass:
    key = (WARMUP, SPLIT_LAST)
    if key not in _program_cache:
        _program_cache[key] = _build_program(WARMUP, SPLIT_LAST)
    return _program_cache[key]


def _prep_inputs(x, conv_w, proj_w, proj_b):
    x = np.asarray(x, dtype=np.float32)
    conv_w = np.asarray(conv_w, dtype=np.float32)
    proj_w = np.asarray(proj_w, dtype=np.float32)
    proj_b = np.asarray(proj_b, dtype=np.float32)

    xT_all = np.zeros((B, D, LP), dtype=np.float32)              # [B, D, L+W-1]
    xT_all[:, :, :L] = x.transpose(0, 2, 1)
    xT_all = np.ascontiguousarray(xT_all.reshape(B, C, P, LP).astype(np.float16))
    cwT = conv_w.transpose(2, 1, 0).reshape(W, C, P, D).astype(np.float16)
    # tap-0 weights additionally pre-split into 36 [128,128] pieces
    cw0s = np.ascontiguousarray(
        cwT[0].reshape(C, P, C, P).transpose(0, 2, 1, 3)
    )                                                            # [c, ob, P, P]
    # batched per-tap layout: cwB[k, p, c*D + o] = conv_w[o, c*128+p, k]
    cwB = np.ascontiguousarray(cwT.transpose(0, 2, 1, 3).reshape(W, P, C * D))
    pwT = proj_w.T.reshape(C, P, D).astype(np.float16)
    pwB = np.ascontiguousarray(pwT.transpose(1, 0, 2).reshape(P, C * D))
    pbb = np.ascontiguousarray(proj_b.reshape(C, P, 1))
    return xT_all, cw0s, cwB, pwB, pbb


def kernel(x, conv_w, proj_w, proj_b):
    global LAST_RESULTS
    nc = _get_program()
    xT_all, cw0s, cwB, pwB, pbb = _prep_inputs(x, conv_w, proj_w, proj_b)
    in_maps = [
        {"xT": xT_all[b], "cw0s": cw0s, "cwB": cwB, "pwB": pwB, "pb": pbb}
        for b in range(B)
    ]
    res = bass_utils.run_bass_kernel_spmd(
        nc, in_maps, core_ids=list(range(B)), trace=TRACE
    )
    LAST_RESULTS = res
    # per-core out is [W, P, C, L] f16; final layout is [L, W, D] with
    # D = c*128 + p
    return np.stack(
        [
            np.ascontiguousarray(
                r["out"].astype(np.float32).transpose(3, 0, 2, 1).reshape(L, W, D)
            )
            for r in res.results
        ],
        axis=0,
    )


# revision 7
# speedup vs baseline: 1.0103x; 1.0026x over previous
"""Trainium2 Bass kernel for nn_ConvShare: multi-width causal conv + shared projection.

Reference computation (per batch element b):
    xpad = pad(x[b], L -> L+W-1)                       # [L+11, D]
    taps[k]  = xpad[k:k+L, :] @ conv_w[:, :, k].T      # [L, D], k = 0..W-1
    spans[k] = cumsum_k taps                           # [L, D]
    h[k]     = relu(spans[k])
    out[:, k, :] = h[k] @ proj_w.T + proj_b            # [L, W, D]

Sharding: data-parallel over batch B=8 across the 8 NeuronCores (no
communication; conv_w/proj_w replicated per core).

Everything on-chip is feature-major ([D, L], contraction dim on SBUF
partitions). fp16 matmul inputs (PSUM accumulation fp32; conv cumsum
carried in fp32 SBUF). The PE roofline is 24 unit-matmuls of
512x768x768 = 184.3us; this implementation closes in on it with:
  - PE warm-up: dummy N=32 matmuls with no DMA deps run during the
    startup input DMA so the HAM clock gate (1.2 -> 2.4 GHz) opens
    before the first real matmul.
  - c-outer conv waves + finely split tap-0 DMAs (xT in 4 partition
    slices, tap-0 conv weights in 36 [128,128] pieces) so the first
    conv wave needs only ~330KB of DMA instead of ~2MB.
  - Batched DMA everywhere else: one 9KB-per-partition transfer per
    conv-weight tap, one 6KB-per-partition transfer per output tap
    (128 lines instead of 768). The DMA engines are line-issue-rate
    limited (~0.1-0.2us per line per queue), not bandwidth limited.
  - fp16 output, transposed/upcast to [L, W, D] host-side.
  - The last tap is emitted in two l-halves and its output DMA in two
    o2-halves so the final drain chain is short.
"""

import os
import sys

import numpy as np

if True:  # make concourse importable regardless of harness cwd
    for _p in ("/opt/trn_rl_repo", "/opt/pypackages"):
        if _p not in sys.path and os.path.isdir(_p):
            sys.path.append(_p)

from contextlib import ExitStack  # noqa: E402

import concourse.bacc as bacc  # noqa: E402
import concourse.bass as bass  # noqa: E402
import concourse.mybir as mybir  # noqa: E402
import concourse.tile as tile  # noqa: E402
from concourse import bass_utils  # noqa: E402

B, L, D, W = 8, 512, 768, 12
P = 128          # SBUF partitions
C = D // P       # 6 contraction chunks of 128
LP = L + W - 1   # 523: right-padded sequence length

F32 = mybir.dt.float32
F16 = mybir.dt.float16
RELU = mybir.ActivationFunctionType.Relu

WARMUP = 64      # dummy N=32 matmuls to open the HAM clock gate during startup DMA
SPLIT_LAST = True  # emit tap W-1 in two l-halves for a faster tail drain

# Knobs the test harness may flip before calling kernel():
TRACE = False
LAST_RESULTS = None


def _build_program(warmup: int, split_last: bool) -> bass.Bass:
    mdt = F16

    nc = bacc.Bacc(
        "TRN2",
        target_bir_lowering=False,
        debug=False,
        num_devices=B,
    )

    # DRAM I/O (pre-arranged host-side so every DMA is a clean copy with
    # large contiguous per-partition lines).
    xT = nc.dram_tensor("xT", [C, P, LP], mdt, kind="ExternalInput").ap()
    cw0s = nc.dram_tensor("cw0s", [C, C, P, P], mdt, kind="ExternalInput").ap()
    cwB = nc.dram_tensor("cwB", [W, P, C * D], mdt, kind="ExternalInput").ap()
    pwB = nc.dram_tensor("pwB", [P, C * D], mdt, kind="ExternalInput").ap()
    pb = nc.dram_tensor("pb", [C, P, 1], F32, kind="ExternalInput").ap()
    out = nc.dram_tensor("out", [W, P, C, L], F16, kind="ExternalOutput").ap()

    with tile.TileContext(nc) as tc, ExitStack() as ctx:
        const_pool = ctx.enter_context(tc.tile_pool(name="const", bufs=1))
        cw_pool = ctx.enter_context(tc.tile_pool(name="cw", bufs=2))
        h_pool = ctx.enter_context(tc.tile_pool(name="h", bufs=2))
        out_pool = ctx.enter_context(tc.tile_pool(name="out", bufs=2))
        psc_pool = ctx.enter_context(tc.tile_pool(name="psc", bufs=1, space="PSUM"))
        psp_pool = ctx.enter_context(tc.tile_pool(name="psp", bufs=2, space="PSUM"))

        # --- PE warm-up: no-data-dependency matmuls that run while the first
        # input DMAs are in flight, so the HAM gate opens (~3.4us of PE busy)
        # before the real stream starts.
        if warmup:
            wa = const_pool.tile([P, P], mdt, name="warm_a")
            wb = const_pool.tile([P, 512], mdt, name="warm_b")
            nc.gpsimd.memset(wa[:], 0.0)
            nc.gpsimd.memset(wb[:], 0.0)
            # short MMs to accumulate ~2.5us of PE-busy quickly, then a few
            # N=512 ones to stay busy until the first real matmul's inputs
            # land (a >3.4us idle gap would re-throttle the HAM gate).
            for wi in range(warmup):
                wp = psp_pool.tile([P, 512], F32, tag="psp", name=f"warm_ps{wi}")
                nn = 32 if wi < warmup - 10 else 512
                nc.tensor.matmul(
                    wp[:, 0:nn], lhsT=wa[:], rhs=wb[:, 0:nn], start=True, stop=True
                )

        # --- running conv cumsum, fp32 (memset overlaps startup DMA)
        spans = const_pool.tile([P, C * L], F32)
        nc.gpsimd.memset(spans[:], 0.0)

        # --- startup DMAs in "wave" order: the tap-0 conv runs c-outer, so
        # wave c needs only xT[c] (4 partition-sliced DMAs) + 6 small weight
        # pieces. pw/pb/cw[1] follow after tap-0's needs.
        xT_t = []
        cw0_t = []  # [c][ob] -> [128,128] tile
        for c in range(C):
            xt = const_pool.tile([P, LP], mdt, tag=f"xt{c}", name=f"xt{c}")
            for p0 in range(0, P, 32):
                nc.sync.dma_start(xt[p0 : p0 + 32, :], xT[c, p0 : p0 + 32, :])
            xT_t.append(xt)
            obs = []
            for ob in range(C):
                t = const_pool.tile([P, P], mdt, tag=f"cw0_{c}_{ob}", name=f"cw0_{c}_{ob}")
                nc.sync.dma_start(t[:], cw0s[c, ob, :, :])
                obs.append(t)
            cw0_t.append(obs)

        def load_cw(k):
            # one DMA per tap: 128 lines of 9KB
            t = cw_pool.tile([P, C * D], mdt, tag="cw", name=f"cw_{k}")
            nc.sync.dma_start(t[:], cwB[k, :, :])
            return t

        # conv psum banks: one persistent tag per output block (6 banks; the
        # remaining 2 banks cycle for the proj).
        ps = [
            psc_pool.tile([P, L], F32, tag=f"sp{ob}", name=f"sp{ob}")
            for ob in range(C)
        ]

        def conv_tap(k, cw_k, l0, ln):
            # wave order: all 6 output blocks for contraction chunk c, then c+1.
            for c in range(C):
                for ob in range(C):
                    lhsT = (
                        cw0_t[c][ob][:]
                        if k == 0
                        else cw_k[:, c * D + ob * P : c * D + (ob + 1) * P]
                    )
                    nc.tensor.matmul(
                        ps[ob][:, l0 : l0 + ln],
                        lhsT=lhsT,
                        rhs=xT_t[c][:, k + l0 : k + l0 + ln],
                        start=(c == 0),
                        stop=(c == C - 1),
                        skip_group_check=True,
                    )

        def relu_tap(h_t, l0, ln):
            for ob in range(C):
                sp = spans[:, ob * L + l0 : ob * L + l0 + ln]
                nc.vector.tensor_add(sp, sp, ps[ob][:, l0 : l0 + ln])  # cumsum
                nc.scalar.activation(h_t[ob][:, l0 : l0 + ln], sp, RELU)

        def proj_tap(k, h_t, pw_t, pb_t, o_tap, l0, ln):
            for o2b in range(C):
                pp = psp_pool.tile([P, 512], F32, tag="psp", name=f"pp_{k}_{o2b}_{l0}")
                for c in range(C):
                    nc.tensor.matmul(
                        pp[:, 0:ln],
                        lhsT=pw_t[:, c * D + o2b * P : c * D + (o2b + 1) * P],
                        rhs=h_t[c][:, l0 : l0 + ln],
                        start=(c == 0),
                        stop=(c == C - 1),
                    )
                nc.vector.tensor_scalar_add(
                    o_tap[:, o2b * L + l0 : o2b * L + l0 + ln], pp[:, 0:ln], pb_t[o2b][:]
                )

        # --- tap 0 conv first (its matmuls only need the wave DMAs above) ---
        conv_tap(0, None, 0, L)

        # remaining startup loads, ordered by first use
        cw_cur = load_cw(1)
        pw_t = const_pool.tile([P, C * D], mdt, name="pw")
        nc.sync.dma_start(pw_t[:], pwB[:, :])
        pb_t = []
        for c in range(C):
            t = const_pool.tile([P, 1], F32, tag=f"pb{c}", name=f"pb{c}")
            nc.sync.dma_start(t[:], pb[c, :, :])
            pb_t.append(t)

        h_t = [h_pool.tile([P, L], mdt, tag=f"h{c}", name=f"h{c}_0") for c in range(C)]
        o_tap = out_pool.tile([P, C * L], F16, tag="out", name="o_0")
        relu_tap(h_t, 0, L)
        proj_tap(0, h_t, pw_t, pb_t, o_tap, 0, L)
        nc.sync.dma_start(out[0, :, :, :], o_tap[:])

        pend = None  # (h_t, o_tap) of tap W-2, whose proj is deferred so the
        #              last tap's conv+relu can hide under it on the PE
        for k in range(1, W):
            cw_k = cw_cur
            cw_cur = load_cw(k + 1) if k + 1 < W else None
            h_t = [
                h_pool.tile([P, L], mdt, tag=f"h{c}", name=f"h{c}_{k}")
                for c in range(C)
            ]
            o_tap = out_pool.tile([P, C * L], F16, tag="out", name=f"o_{k}")
            conv_tap(k, cw_k, 0, L)
            relu_tap(h_t, 0, L)
            if split_last and k == W - 2:
                pend = (h_t, o_tap)
                continue
            if pend is not None:
                ph, po = pend
                pend = None
                proj_tap(k - 1, ph, pw_t, pb_t, po, 0, L)
                nc.sync.dma_start(out[k - 1, :, :, :], po[:])
            if split_last and k == W - 1:
                # output in two o2-half DMAs (3KB lines): the first drains
                # while proj groups 3-5 still compute.
                for o2b in range(C):
                    pp = psp_pool.tile([P, 512], F32, tag="psp", name=f"pp_{k}_{o2b}")
                    for c in range(C):
                        nc.tensor.matmul(
                            pp[:],
                            lhsT=pw_t[:, c * D + o2b * P : c * D + (o2b + 1) * P],
                            rhs=h_t[c][:],
                            start=(c == 0),
                            stop=(c == C - 1),
                        )
                    nc.vector.tensor_scalar_add(
                        o_tap[:, o2b * L : (o2b + 1) * L], pp[:], pb_t[o2b][:]
                    )
                    if o2b == 2:
                        nc.sync.dma_start(out[k, :, 0:3, :], o_tap[:, 0 : 3 * L])
                    elif o2b == 5:
                        nc.sync.dma_start(out[k, :, 3:6, :], o_tap[:, 3 * L : 6 * L])
            else:
                proj_tap(k, h_t, pw_t, pb_t, o_tap, 0, L)
                nc.sync.dma_start(out[k, :, :, :], o_tap[:])

    nc.compile()
    return nc


_program_cache: dict = {}


def _get_program() -> bass.Bass:
    key = (WARMUP, SPLIT_LAST)
    if key not in _program_cache:
        _program_cache[key] = _build_program(WARMUP, SPLIT_LAST)
    return _program_cache[key]


def _prep_inputs(x, conv_w, proj_w, proj_b):
    x = np.asarray(x, dtype=np.float32)
    conv_w = np.asarray(conv_w, dtype=np.float32)
    proj_w = np.asarray(proj_w, dtype=np.float32)
    proj_b = np.asarray(proj_b, dtype=np.float32)

    xT_all = np.zeros((B, D, LP), dtype=np.float32)              # [B, D, L+W-1]
    xT_all[:, :, :L] = x.transpose(0, 2, 1)
    xT_all = np.ascontiguousarray(xT_all.reshape(B, C, P, LP).astype(np.float16))
    cwT = conv_w.transpose(2, 1, 0).reshape(W, C, P, D).astype(np.float16)
    # tap-0 weights additionally pre-split into 36 [128,128] pieces
    cw0s = np.ascontiguousarray(
        cwT[0].reshape(C, P, C, P).transpose(0, 2, 1, 3)
    )                                                            # [c, ob, P, P]
    # batched per-tap layout: cwB[k, p, c*D + o] = conv_w[o, c*128+p, k]
    cwB = np.ascontiguousarray(cwT.transpose(0, 2, 1, 3).reshape(W, P, C * D))
    pwT = proj_w.T.reshape(C, P, D).astype(np.float16)
    pwB = np.ascontiguousarray(pwT.transpose(1, 0, 2).reshape(P, C * D))
    pbb = np.ascontiguousarray(proj_b.reshape(C, P, 1))
    return xT_all, cw0s, cwB, pwB, pbb


def kernel(x, conv_w, proj_w, proj_b):
    global LAST_RESULTS
    nc = _get_program()
    xT_all, cw0s, cwB, pwB, pbb = _prep_inputs(x, conv_w, proj_w, proj_b)
    in_maps = [
        {"xT": xT_all[b], "cw0s": cw0s, "cwB": cwB, "pwB": pwB, "pb": pbb}
        for b in range(B)
    ]
    res = bass_utils.run_bass_kernel_spmd(
        nc, in_maps, core_ids=list(range(B)), trace=TRACE
    )
    LAST_RESULTS = res
    # per-core out is [W, P, C, L] f16; final layout is [L, W, D] with
    # D = c*128 + p
    return np.stack(
        [
            np.ascontiguousarray(
                r["out"].astype(np.float32).transpose(3, 0, 2, 1).reshape(L, W, D)
            )
            for r in res.results
        ],
        axis=0,
    )


# revision 11
# speedup vs baseline: 1.1564x; 1.1446x over previous
"""Trainium2 Bass kernel for nn_ConvShare: multi-width causal conv + shared projection.

Reference computation (per batch element b):
    xpad = pad(x[b], L -> L+W-1)                       # [L+11, D]
    taps[k]  = xpad[k:k+L, :] @ conv_w[:, :, k].T      # [L, D], k = 0..W-1
    spans[k] = cumsum_k taps                           # [L, D]
    h[k]     = relu(spans[k])
    out[:, k, :] = h[k] @ proj_w.T + proj_b            # [L, W, D]

Sharding: data-parallel over batch B=8 across the 8 NeuronCores (no
communication; conv_w/proj_w replicated per core).

Everything on-chip is feature-major ([D, L], contraction dim on SBUF
partitions). fp16 matmul inputs (PSUM accumulation fp32; conv cumsum
carried in fp32 SBUF). The PE roofline is 24 unit-matmuls of
512x768x768 = 184.3us; this implementation closes in on it with:
  - PE warm-up: dummy N=32 matmuls with no DMA deps run during the
    startup input DMA so the HAM clock gate (1.2 -> 2.4 GHz) opens
    before the first real matmul.
  - c-outer conv waves + finely split tap-0 DMAs (xT in 4 partition
    slices, tap-0 conv weights in 36 [128,128] pieces) so the first
    conv wave needs only ~330KB of DMA instead of ~2MB.
  - Batched DMA everywhere else: one 9KB-per-partition transfer per
    conv-weight tap, one 6KB-per-partition transfer per output tap
    (128 lines instead of 768). The DMA engines are line-issue-rate
    limited (~0.1-0.2us per line per queue), not bandwidth limited.
  - fp16 output, transposed/upcast to [L, W, D] host-side.
  - The last tap is emitted in two l-halves and its output DMA in two
    o2-halves so the final drain chain is short.
"""

import os
import sys

import numpy as np

if True:  # make concourse importable regardless of harness cwd
    for _p in ("/opt/trn_rl_repo", "/opt/pypackages"):
        if _p not in sys.path and os.path.isdir(_p):
            sys.path.append(_p)

from contextlib import ExitStack  # noqa: E402

import concourse.bacc as bacc  # noqa: E402
import concourse.bass as bass  # noqa: E402
import concourse.mybir as mybir  # noqa: E402
import concourse.tile as tile  # noqa: E402
from concourse import bass_utils  # noqa: E402

B, L, D, W = 8, 512, 768, 12
P = 128          # SBUF partitions
C = D // P       # 6 contraction chunks of 128
LP = L + W - 1   # 523: right-padded sequence length

F32 = mybir.dt.float32
F16 = mybir.dt.float16
RELU = mybir.ActivationFunctionType.Relu

WARMUP = 64      # dummy N=32 matmuls to open the HAM clock gate during startup DMA
SPLIT_LAST = True  # emit tap W-1 in two l-halves for a faster tail drain

# Knobs the test harness may flip before calling kernel():
TRACE = False
LAST_RESULTS = None


def _build_program(warmup: int, split_last: bool) -> bass.Bass:
    mdt = F16

    nc = bacc.Bacc(
        "TRN2",
        target_bir_lowering=False,
        debug=False,
        num_devices=B,
    )

    # DRAM I/O (pre-arranged host-side so every DMA is a clean copy with
    # large contiguous per-partition lines).
    xT = nc.dram_tensor("xT", [C, P, LP], mdt, kind="ExternalInput").ap()
    cw0s = nc.dram_tensor("cw0s", [C, P, D], mdt, kind="ExternalInput").ap()
    cwB = nc.dram_tensor("cwB", [W, P, C * D], mdt, kind="ExternalInput").ap()
    pwB = nc.dram_tensor("pwB", [P, C * D], mdt, kind="ExternalInput").ap()
    pb = nc.dram_tensor("pb", [C, P, 1], F32, kind="ExternalInput").ap()
    out = nc.dram_tensor("out", [W, P, C, L], F16, kind="ExternalOutput").ap()

    with tile.TileContext(nc) as tc, ExitStack() as ctx:
        const_pool = ctx.enter_context(tc.tile_pool(name="const", bufs=1))
        cw_pool = ctx.enter_context(tc.tile_pool(name="cw", bufs=2))
        h_pool = ctx.enter_context(tc.tile_pool(name="h", bufs=2))
        out_pool = ctx.enter_context(tc.tile_pool(name="out", bufs=2))
        psc_pool = ctx.enter_context(tc.tile_pool(name="psc", bufs=1, space="PSUM"))
        psp_pool = ctx.enter_context(tc.tile_pool(name="psp", bufs=2, space="PSUM"))

        # --- PE warm-up: no-data-dependency matmuls that run while the first
        # input DMAs are in flight, so the HAM gate opens (~3.4us of PE busy)
        # before the real stream starts.
        if warmup:
            wa = const_pool.tile([P, P], mdt, name="warm_a")
            wb = const_pool.tile([P, 512], mdt, name="warm_b")
            nc.gpsimd.memset(wa[:], 0.0)
            nc.gpsimd.memset(wb[:], 0.0)
            # short MMs to accumulate ~2.5us of PE-busy quickly, then a few
            # N=512 ones to stay busy until the first real matmul's inputs
            # land (a >3.4us idle gap would re-throttle the HAM gate).
            for wi in range(warmup):
                wp = psp_pool.tile([P, 512], F32, tag="psp", name=f"warm_ps{wi}")
                nn = 32 if wi < warmup - 10 else 512
                nc.tensor.matmul(
                    wp[:, 0:nn], lhsT=wa[:], rhs=wb[:, 0:nn], start=True, stop=True
                )

        # --- running conv cumsum, fp32 (memset overlaps startup DMA)
        spans = const_pool.tile([P, C * L], F32)
        nc.gpsimd.memset(spans[:], 0.0)

        # --- startup DMAs in "wave" order: the tap-0 conv runs c-outer, so
        # wave c needs only xT[c] (4 partition-sliced DMAs) + 6 small weight
        # pieces. pw/pb/cw[1] follow after tap-0's needs.
        xT_t = []
        cw0_t = []  # [c] -> [128, D] tile (tap-0 weights, one DMA per chunk:
        #             keep startup lines at 1-1.5KB — finer splits explode the
        #             DMA line count and crawl for ~50us at ~150ns/line/queue)
        for c in range(C):
            xt = const_pool.tile([P, LP], mdt, tag=f"xt{c}", name=f"xt{c}")
            nc.sync.dma_start(xt[:], xT[c, :, :])
            xT_t.append(xt)
            t = const_pool.tile([P, D], mdt, tag=f"cw0_{c}", name=f"cw0_{c}")
            nc.sync.dma_start(t[:], cw0s[c, :, :])
            cw0_t.append(t)

        def load_cw(k):
            # one DMA per tap: 128 lines of 9KB
            t = cw_pool.tile([P, C * D], mdt, tag="cw", name=f"cw_{k}")
            nc.sync.dma_start(t[:], cwB[k, :, :])
            return t

        # conv psum banks: one persistent tag per output block (6 banks; the
        # remaining 2 banks cycle for the proj).
        ps = [
            psc_pool.tile([P, L], F32, tag=f"sp{ob}", name=f"sp{ob}")
            for ob in range(C)
        ]

        def conv_tap(k, cw_k, l0, ln):
            # wave order: all 6 output blocks for contraction chunk c, then c+1.
            for c in range(C):
                for ob in range(C):
                    lhsT = (
                        cw0_t[c][:, ob * P : (ob + 1) * P]
                        if k == 0
                        else cw_k[:, c * D + ob * P : c * D + (ob + 1) * P]
                    )
                    nc.tensor.matmul(
                        ps[ob][:, l0 : l0 + ln],
                        lhsT=lhsT,
                        rhs=xT_t[c][:, k + l0 : k + l0 + ln],
                        start=(c == 0),
                        stop=(c == C - 1),
                        skip_group_check=True,
                    )

        def relu_tap(h_t, l0, ln):
            for ob in range(C):
                sp = spans[:, ob * L + l0 : ob * L + l0 + ln]
                nc.vector.tensor_add(sp, sp, ps[ob][:, l0 : l0 + ln])  # cumsum
                nc.scalar.activation(h_t[ob][:, l0 : l0 + ln], sp, RELU)

        def proj_tap(k, h_t, pw_t, pb_t, o_tap, l0, ln):
            for o2b in range(C):
                pp = psp_pool.tile([P, 512], F32, tag="psp", name=f"pp_{k}_{o2b}_{l0}")
                for c in range(C):
                    nc.tensor.matmul(
                        pp[:, 0:ln],
                        lhsT=pw_t[:, c * D + o2b * P : c * D + (o2b + 1) * P],
                        rhs=h_t[c][:, l0 : l0 + ln],
                        start=(c == 0),
                        stop=(c == C - 1),
                    )
                nc.vector.tensor_scalar_add(
                    o_tap[:, o2b * L + l0 : o2b * L + l0 + ln], pp[:, 0:ln], pb_t[o2b][:]
                )

        # --- tap 0 conv first (its matmuls only need the wave DMAs above) ---
        conv_tap(0, None, 0, L)

        # remaining startup loads, ordered by first use
        cw_cur = load_cw(1)
        pw_t = const_pool.tile([P, C * D], mdt, name="pw")
        nc.sync.dma_start(pw_t[:], pwB[:, :])
        pb_t = []
        for c in range(C):
            t = const_pool.tile([P, 1], F32, tag=f"pb{c}", name=f"pb{c}")
            nc.sync.dma_start(t[:], pb[c, :, :])
            pb_t.append(t)

        h_t = [h_pool.tile([P, L], mdt, tag=f"h{c}", name=f"h{c}_0") for c in range(C)]
        o_tap = out_pool.tile([P, C * L], F16, tag="out", name="o_0")
        relu_tap(h_t, 0, L)
        proj_tap(0, h_t, pw_t, pb_t, o_tap, 0, L)
        nc.sync.dma_start(out[0, :, :, :], o_tap[:])

        pend = None  # (h_t, o_tap) of tap W-2, whose proj is deferred so the
        #              last tap's conv+relu can hide under it on the PE
        for k in range(1, W):
            cw_k = cw_cur
            cw_cur = load_cw(k + 1) if k + 1 < W else None
            h_t = [
                h_pool.tile([P, L], mdt, tag=f"h{c}", name=f"h{c}_{k}")
                for c in range(C)
            ]
            o_tap = out_pool.tile([P, C * L], F16, tag="out", name=f"o_{k}")
            conv_tap(k, cw_k, 0, L)
            relu_tap(h_t, 0, L)
            if split_last and k == W - 2:
                pend = (h_t, o_tap)
                continue
            if pend is not None:
                ph, po = pend
                pend = None
                proj_tap(k - 1, ph, pw_t, pb_t, po, 0, L)
                nc.sync.dma_start(out[k - 1, :, :, :], po[:])
            if split_last and k == W - 1:
                # output in two o2-half DMAs (3KB lines): the first drains
                # while proj groups 3-5 still compute.
                for o2b in range(C):
                    pp = psp_pool.tile([P, 512], F32, tag="psp", name=f"pp_{k}_{o2b}")
                    for c in range(C):
                        nc.tensor.matmul(
                            pp[:],
                            lhsT=pw_t[:, c * D + o2b * P : c * D + (o2b + 1) * P],
                            rhs=h_t[c][:],
                            start=(c == 0),
                            stop=(c == C - 1),
                        )
                    nc.vector.tensor_scalar_add(
                        o_tap[:, o2b * L : (o2b + 1) * L], pp[:], pb_t[o2b][:]
                    )
                    if o2b == 2:
                        nc.sync.dma_start(out[k, :, 0:3, :], o_tap[:, 0 : 3 * L])
                    elif o2b == 5:
                        nc.sync.dma_start(out[k, :, 3:6, :], o_tap[:, 3 * L : 6 * L])
            else:
                proj_tap(k, h_t, pw_t, pb_t, o_tap, 0, L)
                nc.sync.dma_start(out[k, :, :, :], o_tap[:])

    nc.compile()
    return nc


_program_cache: dict = {}


def _get_program() -> bass.Bass:
    key = (WARMUP, SPLIT_LAST)
    if key not in _program_cache:
        _program_cache[key] = _build_program(WARMUP, SPLIT_LAST)
    return _program_cache[key]


def _prep_inputs(x, conv_w, proj_w, proj_b):
    x = np.asarray(x, dtype=np.float32)
    conv_w = np.asarray(conv_w, dtype=np.float32)
    proj_w = np.asarray(proj_w, dtype=np.float32)
    proj_b = np.asarray(proj_b, dtype=np.float32)

    xT_all = np.zeros((B, D, LP), dtype=np.float32)              # [B, D, L+W-1]
    xT_all[:, :, :L] = x.transpose(0, 2, 1)
    xT_all = np.ascontiguousarray(xT_all.reshape(B, C, P, LP).astype(np.float16))
    cwT = conv_w.transpose(2, 1, 0).reshape(W, C, P, D).astype(np.float16)
    # tap-0 weights separately in per-chunk layout (startup waves)
    cw0s = np.ascontiguousarray(cwT[0])                          # [C, P, D]
    # batched per-tap layout: cwB[k, p, c*D + o] = conv_w[o, c*128+p, k]
    cwB = np.ascontiguousarray(cwT.transpose(0, 2, 1, 3).reshape(W, P, C * D))
    pwT = proj_w.T.reshape(C, P, D).astype(np.float16)
    pwB = np.ascontiguousarray(pwT.transpose(1, 0, 2).reshape(P, C * D))
    pbb = np.ascontiguousarray(proj_b.reshape(C, P, 1))
    return xT_all, cw0s, cwB, pwB, pbb


def kernel(x, conv_w, proj_w, proj_b):
    global LAST_RESULTS
    nc = _get_program()
    xT_all, cw0s, cwB, pwB, pbb = _prep_inputs(x, conv_w, proj_w, proj_b)
    in_maps = [
        {"xT": xT_all[b], "cw0s": cw0s, "cwB": cwB, "pwB": pwB, "pb": pbb}
        for b in range(B)
    ]
    res = bass_utils.run_bass_kernel_spmd(
        nc, in_maps, core_ids=list(range(B)), trace=TRACE
    )
    LAST_RESULTS = res
    # per-core out is [W, P, C, L] f16; final layout is [L, W, D] with
    # D = c*128 + p
    return np.stack(
        [
            np.ascontiguousarray(
                r["out"].astype(np.float32).transpose(3, 0, 2, 1).reshape(L, W, D)
            )
            for r in res.results
        ],
        axis=0,
    )
